# revision 33
# baseline (speedup 1.0000x reference)
"""Trainium2 Bass kernel for grouped-expert 3-layer MLP (MoE) — expert-parallel.

Sharding: expert-parallel across 8 NeuronCores (E=16 -> 2 experts/core, full
batch B=8192 per core). Host pre-marshals each core's inputs into the exact
device layouts (pure layout/dtype marshalling): x-shard feature-major bf16
[2, DIN, B], weights bf16 pre-transposed to SBUF tile layouts, biases
pre-transposed/summed f32. Device computes partial sums over its 2 experts as
out_c[DOUT, B]; host unshards by summing the 8 partials and transposing.

Per-core pipeline (bf16 matmuls, feature-major activations, nb=512 tiles,
batch tiles processed in PAIRS for layer-3 PE column-tiling):
  h1[hb] = relu(W1[:,hb].T @ x + b1)          4 blocks of [128, nb] per (bt,e)
  h2[gb] = relu(sum_hb W2[hb,gb].T @ h1 + b2)
  L3 pair (a,b): po[0:64]  += sum_{e,gb} W3[gb].T @ h2_a   (PE col group h0)
                 po[64:128]+= sum_{e,gb} W3[gb].T @ h2_b   (PE col group h1)
     interleaved A/B so both column groups stream concurrently (2x L3).
  epilogue: ACT/DVE add summed b3 during PSUM->SBUF evac, DMA both halves to
  out[DOUT, B] feature-major. Last pair splits L3 into uneven column chunks
  (384/128) so the final evac+store overlaps the last matmuls.

DMA startup order is tuned for the ~2us queue latency + round-robin fabric
(~300GB/s aggregate on these small-packet DMAs): per-queue dispatch
serialization (~0.65us each) enforces need-order — x(0,*)/W1/biases first,
W2 blocks next, W3 + late x tiles on gpsimd. The scalar (ACT) queue carries
only 3 dispatches so the first L1 evacuations are not delayed.

No device-side casts or transposes; weights loaded once, SBUF-resident.
"""

import os
from contextlib import ExitStack

import ml_dtypes
import numpy as np

import concourse.bass as bass
import concourse.tile as tile
from concourse import bacc, mybir
from concourse.bass_utils import run_bass_kernel_spmd

E_TOT, DIN, H, DOUT = 16, 128, 512, 64
B_FULL = 8192
N_CORES = 8
E_LOC = E_TOT // N_CORES  # 2 experts per core
HB = H // 128  # 4 h-blocks
F32 = mybir.dt.float32
BF = mybir.dt.bfloat16
BF_NP = ml_dtypes.bfloat16


def build_nc(nb=512):
    B = B_FULL
    nbt = B // nb  # 16 batch tiles
    npair = nbt // 2
    nc = bacc.Bacc("TRN2", target_bir_lowering=False, debug=False)

    xt = nc.dram_tensor("xt", [E_LOC, DIN, B], BF, kind="ExternalInput")
    w1 = nc.dram_tensor("w1", [DIN, E_LOC, H], BF, kind="ExternalInput")
    w2 = nc.dram_tensor("w2", [128, E_LOC, HB, H], BF, kind="ExternalInput")
    w3 = nc.dram_tensor("w3", [128, E_LOC, HB, DOUT], BF, kind="ExternalInput")
    # all biases in one tensor -> one DMA dispatch: [b1s | b2s | b3sum]
    bc = nc.dram_tensor("bc", [128, 2 * HB * E_LOC + 1], F32, kind="ExternalInput")
    out = nc.dram_tensor("out", [B // 512, DOUT, 512], F32, kind="ExternalOutput")

    RELU = mybir.ActivationFunctionType.Relu
    IDENT = mybir.ActivationFunctionType.Identity
    ADD = mybir.AluOpType.add
    MAX = mybir.AluOpType.max

    with tile.TileContext(nc) as tc, ExitStack() as ctx:
        consts = ctx.enter_context(tc.tile_pool(name="consts", bufs=1))
        wp = ctx.enter_context(tc.tile_pool(name="wp", bufs=1))
        xtp = ctx.enter_context(tc.tile_pool(name="xtp", bufs=4))
        h1p = ctx.enter_context(tc.tile_pool(name="h1p", bufs=6))
        h2p = ctx.enter_context(tc.tile_pool(name="h2p", bufs=6))
        obp = ctx.enter_context(tc.tile_pool(name="obp", bufs=3))
        p1p = ctx.enter_context(tc.tile_pool(name="p1p", bufs=4, space="PSUM"))
        p2p = ctx.enter_context(tc.tile_pool(name="p2p", bufs=3, space="PSUM"))
        pop = ctx.enter_context(tc.tile_pool(name="pop", bufs=1, space="PSUM"))

        def emit_x_dma(bt, e, eng=None):
            b0 = bt * nb
            xtt = xtp.tile([DIN, nb], BF, tag="xt")
            (eng or nc.gpsimd).dma_start(out=xtt, in_=xt[e, :, b0 : b0 + nb])
            return xtt

        # ---- critical first DMA wave (everything bf16 -> rides HWDGE) ----
        # The fabric round-robins all in-flight transfers (~300GB/s on these
        # small-packet DMAs), so ordering is enforced by per-queue dispatch
        # serialization (~0.65us per dispatch): need-order on sync+scalar,
        # nothing on gpsimd at startup.
        x_pre = {}
        w1t = wp.tile([DIN, E_LOC, H], BF)
        w2t = wp.tile([128, E_LOC, HB, H], BF)
        w3t = wp.tile([128, E_LOC, HB, DOUT], BF)
        bct = consts.tile([128, 2 * HB * E_LOC + 1], F32)
        B2OFF = HB * E_LOC  # b2 column offset in bct
        B3OFF = 2 * HB * E_LOC  # b3 column offset in bct
        x_pre[(0, 0)] = emit_x_dma(0, 0, nc.sync)
        x_pre[(0, 1)] = emit_x_dma(0, 1, nc.scalar)
        nc.sync.dma_start(out=w1t, in_=w1[:, :, :])
        nc.scalar.dma_start(out=bct, in_=bc[:, :])
        x_pre[(1, 1)] = emit_x_dma(1, 1, nc.gpsimd)
        nc.gpsimd.dma_start(out=w3t, in_=w3[:, :, :, :])
        nc.sync.dma_start(out=w2t[:, 0, 0:2, :], in_=w2[:, 0, 0:2, :])
        nc.scalar.dma_start(out=w2t[:, 0, 2:4, :], in_=w2[:, 0, 2:4, :])
        x_pre[(1, 0)] = emit_x_dma(1, 0, nc.sync)
        nc.sync.dma_start(out=w2t[:, 1, 0:2, :], in_=w2[:, 1, 0:2, :])
        nc.sync.dma_start(out=w2t[:, 1, 2:4, :], in_=w2[:, 1, 2:4, :])

        # warm the PE clock gate while the first wave is in flight (wrm via
        # the otherwise-idle DVE so warmups start immediately)
        wrm = consts.tile([128, 512], BF)
        nc.vector.memset(wrm, 1.0)
        pwu = pop.tile([128, nb], F32, tag="po", name="pwu")
        for _ in range(11):
            nc.tensor.matmul(pwu, wrm[:, 0:128], wrm, start=True, stop=True)

        def emit_l1(bt, e, xtt=None, split_evac=False):
            if xtt is None:
                xtt = emit_x_dma(bt, e)
            h1 = h1p.tile([128, HB, nb], BF, tag="h1")
            for hb in range(HB):
                ps = p1p.tile([128, nb], F32, tag="p1")
                nc.tensor.matmul(
                    ps,
                    w1t[:, e, hb * 128 : (hb + 1) * 128],
                    xtt,
                    start=True,
                    stop=True,
                )
                bias = bct[:, hb * E_LOC + e : hb * E_LOC + e + 1]
                if split_evac:
                    # prologue: free p1p banks ~2x sooner (both engines work
                    # column halves) so the bunched L1 sets don't stall
                    hn = nb // 2
                    nc.scalar.activation(h1[:, hb, 0:hn], ps[:, 0:hn], RELU, bias=bias)
                    nc.vector.tensor_scalar(
                        h1[:, hb, hn:nb], ps[:, hn:nb], bias, 0.0, ADD, MAX
                    )
                elif hb in (0, 3):
                    nc.scalar.activation(h1[:, hb, :], ps, RELU, bias=bias)
                else:
                    nc.vector.tensor_scalar(h1[:, hb, :], ps, bias, 0.0, ADD, MAX)
            return h1

        def emit_l2(e, h1, split_evac=False):
            h2 = h2p.tile([128, HB, nb], BF, tag="h2")
            for gb in range(HB):
                ps = p2p.tile([128, nb], F32, tag="p2")
                for hb in range(HB):
                    nc.tensor.matmul(
                        ps,
                        w2t[:, e, hb, gb * 128 : (gb + 1) * 128],
                        h1[:, hb, :],
                        start=(hb == 0),
                        stop=(hb == HB - 1),
                    )
                bias = bct[:, B2OFF + gb * E_LOC + e : B2OFF + gb * E_LOC + e + 1]
                if split_evac:
                    # land h2 sooner for the L3 B-group: halves on both engines
                    hn = nb // 2
                    nc.scalar.activation(
                        h2[:, gb, 0:hn], ps[:, 0:hn], RELU, bias=bias
                    )
                    nc.vector.tensor_scalar(
                        h2[:, gb, hn:nb], ps[:, hn:nb], bias, 0.0, ADD, MAX
                    )
                elif gb in (0, 3):
                    nc.scalar.activation(h2[:, gb, :], ps, RELU, bias=bias)
                else:
                    nc.vector.tensor_scalar(h2[:, gb, :], ps, bias, 0.0, ADD, MAX)
            return h2

        # prologue: L1 for the first pair (x DMAs already dispatched above)
        h1a = {e: emit_l1(0, e, x_pre[(0, e)], split_evac=True) for e in range(E_LOC)}
        h1b = {e: emit_l1(1, e, x_pre[(1, e)], split_evac=True) for e in range(E_LOC)}

        for pt in range(npair):
            a0 = (2 * pt) * nb
            b0 = (2 * pt + 1) * nb
            last = pt + 1 == npair
            h2a, h2b, h1an, h1bn = {}, {}, {}, {}
            h2a[0] = emit_l2(0, h1a[0])
            if not last:
                h1an[0] = emit_l1(2 * pt + 2, 0)
            h2a[1] = emit_l2(1, h1a[1])
            if not last:
                h1an[1] = emit_l1(2 * pt + 2, 1)
            h2b[0] = emit_l2(0, h1b[0])
            if not last:
                h1bn[0] = emit_l1(2 * pt + 3, 0)
            h2b[1] = emit_l2(1, h1b[1], split_evac=True)
            if not last:
                h1bn[1] = emit_l1(2 * pt + 3, 1)
            h1a, h1b = h1an, h1bn

            # ---- layer 3: both pair members concurrently on PE col groups ----
            ob = obp.tile([128, nb], F32, tag="ob")
            b3ap = bct[:, B3OFF : B3OFF + 1]

            def l3_chunk(po_t, pc0, c0, c1):
                # po_t columns pc0.. hold batch columns c0..c1 of the pair
                idx = 0
                for e in range(E_LOC):
                    for gb in range(HB):
                        idx += 1
                        st = idx == 1
                        sp = idx == E_LOC * HB
                        nc.tensor.matmul(
                            po_t[0:DOUT, pc0 : pc0 + c1 - c0],
                            w3t[:, e, gb, :],
                            h2a[e][:, gb, c0:c1],
                            start=st,
                            stop=sp,
                            skip_group_check=True,
                        )
                        nc.tensor.matmul(
                            po_t[DOUT : 2 * DOUT, pc0 : pc0 + c1 - c0],
                            w3t[:, e, gb, :],
                            h2b[e][:, gb, c0:c1],
                            start=st,
                            stop=sp,
                            skip_group_check=True,
                        )

            # both pair members stored by ONE DMA: out viewed as
            # [(tile o) c] rows 2pt*64..2pt*64+128 = (a rows, b rows)
            out_pair = out[2 * pt : 2 * pt + 2, :, :]
            if not last:
                po = pop.tile([128, nb], F32, tag="po")
                l3_chunk(po, 0, 0, nb)
                # bias add during evac (ACT low cols, DVE high cols), then one
                # store for the whole pair on the otherwise-idle sync queue
                hn = nb // 2
                nc.scalar.activation(ob[:, 0:hn], po[:, 0:hn], IDENT, bias=b3ap)
                nc.vector.tensor_scalar_add(ob[:, hn:nb], po[:, hn:nb], b3ap)
                nc.sync.dma_start(out=out_pair, in_=ob[:, :])
            else:
                # last pair: uneven column chunks (384 + 128) with the short
                # chunk in its own PSUM tile so its matmuls don't wait on the
                # big chunk's evacuation (tile-granular WAR tracking)
                po = pop.tile([128, nb], F32, tag="po")
                po2 = p2p.tile([128, nb - 384], F32, tag="p2")
                l3_chunk(po, 0, 0, 384)
                l3_chunk(po2, 0, 384, nb)
                nc.scalar.activation(ob[:, 0:192], po[:, 0:192], IDENT, bias=b3ap)
                nc.vector.tensor_scalar_add(ob[:, 192:384], po[:, 192:384], b3ap)
                nc.sync.dma_start(out=out_pair[:, :, 0:384], in_=ob[:, 0:384])
                nc.scalar.activation(ob[:, 384:nb], po2, IDENT, bias=b3ap)
                nc.scalar.dma_start(out=out_pair[:, :, 384:nb], in_=ob[:, 384:nb])

    nc.compile()
    return nc


_NC_CACHE = {}


def _get_nc():
    if "nc" not in _NC_CACHE:
        _NC_CACHE["nc"] = build_nc()
    return _NC_CACHE["nc"]


def make_in_maps(x, W1, b1, W2, b2, W3, b3):
    """Host-side marshalling: slice per core and pre-arrange into the device
    layouts (pure layout/dtype work, no math beyond the b3 expert-sum)."""
    in_maps = []
    for c in range(N_CORES):
        sl = slice(E_LOC * c, E_LOC * (c + 1))
        xc = np.ascontiguousarray(x[:, sl, :].transpose(1, 2, 0)).astype(BF_NP)
        w1c = np.ascontiguousarray(W1[sl].transpose(1, 0, 2)).astype(BF_NP)
        w2c = np.ascontiguousarray(
            W2[sl].reshape(E_LOC, HB, 128, H).transpose(2, 0, 1, 3)
        ).astype(BF_NP)
        w3c = np.ascontiguousarray(
            W3[sl].reshape(E_LOC, HB, 128, DOUT).transpose(2, 0, 1, 3)
        ).astype(BF_NP)
        b1c = b1[sl].reshape(E_LOC, HB, 128).transpose(2, 1, 0).reshape(128, HB * E_LOC)
        b2c = b2[sl].reshape(E_LOC, HB, 128).transpose(2, 1, 0).reshape(128, HB * E_LOC)
        b3v = b3[sl].sum(0).astype(np.float32)
        b3c = np.concatenate([b3v, b3v]).reshape(128, 1)
        bcc = np.ascontiguousarray(
            np.concatenate([b1c, b2c, b3c], axis=1).astype(np.float32)
        )
        in_maps.append({"xt": xc, "w1": w1c, "w2": w2c, "w3": w3c, "bc": bcc})
    return in_maps


def kernel(x, W1, b1, W2, b2, W3, b3):
    x = np.asarray(x, dtype=np.float32)
    W1 = np.asarray(W1, dtype=np.float32)
    b1 = np.asarray(b1, dtype=np.float32)
    W2 = np.asarray(W2, dtype=np.float32)
    b2 = np.asarray(b2, dtype=np.float32)
    W3 = np.asarray(W3, dtype=np.float32)
    b3 = np.asarray(b3, dtype=np.float32)

    nc = _get_nc()
    in_maps = make_in_maps(x, W1, b1, W2, b2, W3, b3)
    trace = bool(int(os.environ.get("KERNEL_TRACE", "0")))
    kwargs = {}
    if trace and os.environ.get("KERNEL_TRACE_DIR"):
        kwargs["tmpdir"] = os.environ["KERNEL_TRACE_DIR"]
    res = run_bass_kernel_spmd(nc, in_maps, list(range(N_CORES)), trace=trace, **kwargs)
    if trace:
        kernel.last_results = res
    acc = res.results[0]["out"].astype(np.float64)
    for c in range(1, N_CORES):
        acc += res.results[c]["out"]
    # device layout [tile, DOUT, col] -> [B, DOUT]
    return np.ascontiguousarray(
        acc.transpose(0, 2, 1).reshape(B_FULL, DOUT).astype(np.float32)
    )
